# revision 9
# baseline (speedup 1.0000x reference)
"""Expert-parallel MoE FFN kernel for Trainium2 (8 NeuronCores).

Reference computation (per expert e):
    y[:, e*C:(e+1)*C, :] = gelu(x_e @ w1[e] + b1[e]) @ w2[e] + b2[e]

Sharding: expert-parallel — core e owns expert e (E == n_cores == 8) and the
matching chunk of dim 1 of `inputs`. No cross-core communication.

Per-core dataflow (T=16384 tokens, D=512, F=2048), all matmuls bf16:
  - X is pre-transposed and pre-cast to bf16 on the HOST into
    [128d, nchunk, DSUB, 512t] so the device streams X^T tiles straight from
    DRAM (no on-device cast, no DRAM bounce, no XBAR transpose). This pulls
    the first matmul from t=35us down to the preamble floor and removes
    ~48 MiB/core of HBM traffic.
  - mm1: H^T[f, t] += W1[d, f].T @ X^T[d, t]; gelu+b1 fused on ScalarE
    (f on partitions -> b1 is a per-partition bias), H stored bf16.
  - mm2: Y[t, d] += (H^T[f, t128]).T @ W2[f, d] with H^T as the stationary
    operand, so Y comes out token-major and stores contiguously.
"""

import numpy as np
import ml_dtypes

import concourse.bacc as bacc
import concourse.bass as bass
import concourse.mybir as mybir
import concourse.tile as tile
from concourse.bass_utils import run_bass_kernel_spmd

B, EC, D = 16, 8192, 512
E, F = 8, 2048
C = EC // E            # capacity per expert = 1024
T = B * C              # tokens per expert/core = 16384
P = 128
DSUB = D // P          # 4
FSUB = F // P          # 16
TCHUNK = 512
TS = TCHUNK // P       # 4
NCHUNK = T // TCHUNK   # 32
N_CORES = 8

# Stash of the last BassKernelResults (for test harness profiling).
LAST_RESULT = None


def build_nc(n_tokens: int = T, act_func=None):
    if act_func is None:
        act_func = mybir.ActivationFunctionType.Gelu_apprx_tanh
    nchunk = n_tokens // TCHUNK
    nc = bacc.Bacc(
        "TRN2",
        target_bir_lowering=False,
        debug=False,
        num_devices=N_CORES,
    )
    # Host-pre-transposed X^T: xt[p, c, ds, j] = x[c*512 + j, ds*128 + p], bf16
    xt_d = nc.dram_tensor(
        "xt", [P, nchunk, DSUB, TCHUNK], mybir.dt.bfloat16, kind="ExternalInput"
    ).ap()
    # fs-major w1 so each 128x128 lhsT tile is contiguous per partition and
    # the first f-tile can be DMA'd ahead of the bulk.
    w1 = nc.dram_tensor(
        "w1", [P, FSUB, DSUB, P], mybir.dt.bfloat16, kind="ExternalInput"
    ).ap()
    b1 = nc.dram_tensor("b1", [P, FSUB], mybir.dt.float32, kind="ExternalInput").ap()
    w2 = nc.dram_tensor("w2", [P, FSUB, D], mybir.dt.bfloat16, kind="ExternalInput").ap()
    b2 = nc.dram_tensor("b2", [P, D], mybir.dt.float32, kind="ExternalInput").ap()
    y = nc.dram_tensor("y", [n_tokens, D], mybir.dt.float32, kind="ExternalOutput").ap()

    with tile.TileContext(nc) as tc:
        with (
            tc.tile_pool(name="consts", bufs=1) as consts,
            tc.tile_pool(name="xt", bufs=4) as xt_pool,
            tc.tile_pool(name="h", bufs=2) as h_pool,
            tc.tile_pool(name="yout", bufs=4) as y_pool,
            tc.tile_pool(name="ps_h", bufs=4, space="PSUM") as ps_h,
            tc.tile_pool(name="ps_y", bufs=4, space="PSUM") as ps_y,
        ):
            # Spread const loads over parallel HWDGE queues so the first mm1
            # weight tile lands ~1us after the preamble instead of ~12us:
            #   scalar: b1 + w1 (first f-tile alone, then the bulk)
            #   gpsimd: b2 + w2 (needed only once mm2 starts, ~14us in) + Y
            #   sync:   X^T chunk stream.
            b1_sb = consts.tile([P, FSUB], mybir.dt.float32)
            nc.scalar.dma_start(b1_sb, b1)
            w1_sb = consts.tile([P, FSUB, DSUB, P], mybir.dt.bfloat16)
            nc.scalar.dma_start(w1_sb[:, 0], w1[:, 0])
            for lo, hi in ((1, 6), (6, 11), (11, 16)):
                nc.scalar.dma_start(w1_sb[:, lo:hi], w1[:, lo:hi])
            b2_sb = consts.tile([P, D], mybir.dt.float32)
            nc.gpsimd.dma_start(b2_sb, b2)
            w2_sb = consts.tile([P, FSUB, D], mybir.dt.bfloat16)
            for i in range(4):
                nc.gpsimd.dma_start(
                    w2_sb[:, i * 4:(i + 1) * 4, :], w2[:, i * 4:(i + 1) * 4, :]
                )

            for c in range(nchunk):
                # X^T [d, t] bf16 straight from DRAM (4 KiB/partition, contig).
                xt = xt_pool.tile([P, DSUB, TCHUNK], mybir.dt.bfloat16)
                nc.sync.dma_start(xt, xt_d[:, c])

                # mm1 + fused gelu/bias: H^T[f, t] bf16.
                h = h_pool.tile([P, FSUB, TCHUNK], mybir.dt.bfloat16)
                for fs in range(FSUB):
                    ph = ps_h.tile([P, TCHUNK], mybir.dt.float32)
                    for ds in range(DSUB):
                        nc.tensor.matmul(
                            ph,
                            lhsT=w1_sb[:, fs, ds, :],
                            rhs=xt[:, ds, :],
                            start=(ds == 0),
                            stop=(ds == DSUB - 1),
                        )
                    nc.scalar.activation(
                        h[:, fs, :],
                        ph,
                        act_func,
                        bias=b1_sb[:, fs:fs + 1],
                        scale=1.0,
                    )

                # mm2: Y[t, d] per 128-token subtile; + b2; store.
                for ts in range(TS):
                    py = ps_y.tile([P, D], mybir.dt.float32)
                    for fs in range(FSUB):
                        nc.tensor.matmul(
                            py,
                            lhsT=h[:, fs, ts * P:(ts + 1) * P],
                            rhs=w2_sb[:, fs, :],
                            start=(fs == 0),
                            stop=(fs == FSUB - 1),
                        )
                    y_sb = y_pool.tile([P, D], mybir.dt.float32)
                    nc.vector.tensor_add(y_sb, py, b2_sb)
                    r0 = c * TCHUNK + ts * P
                    nc.gpsimd.dma_start(y[r0:r0 + P, :], y_sb)

    nc.compile()
    return nc


_NC_CACHE = {}


def _get_nc(n_tokens: int = T):
    if n_tokens not in _NC_CACHE:
        _NC_CACHE[n_tokens] = build_nc(n_tokens)
    return _NC_CACHE[n_tokens]


def make_in_maps(inputs, w1, b1, w2, b2):
    """Shard + lay out host-side: core e gets expert e."""
    bf16 = ml_dtypes.bfloat16
    inputs = np.asarray(inputs)
    w1, b1 = np.asarray(w1), np.asarray(b1)
    w2, b2 = np.asarray(w2), np.asarray(b2)
    in_maps = []
    for e in range(E):
        x_e = np.asarray(
            inputs[:, e * C:(e + 1) * C, :], dtype=np.float32
        ).reshape(T, D)
        # X^T bf16: xt[p, c, ds, j] = x_e[c*TCHUNK + j, ds*128 + p]
        xt_e = np.ascontiguousarray(
            x_e.reshape(NCHUNK, TCHUNK, DSUB, P).transpose(3, 0, 2, 1).astype(bf16)
        )
        # w1[e] [D, F] -> [P, FSUB, DSUB, P] with d = ds*128 + p, f = fs*128 + j
        w1_e = np.ascontiguousarray(
            w1[e].reshape(DSUB, P, FSUB, P).transpose(1, 2, 0, 3).astype(bf16)
        )
        # b1[e] [F] -> [P, FSUB] with f = fs*128 + p
        b1_e = np.ascontiguousarray(
            b1[e].reshape(FSUB, P).T.astype(np.float32)
        )
        # w2[e] [F, D] -> [P, FSUB, D] with f = fs*128 + p
        w2_e = np.ascontiguousarray(
            w2[e].reshape(FSUB, P, D).transpose(1, 0, 2).astype(bf16)
        )
        # b2[e] [D] -> broadcast to [P, D]
        b2_e = np.ascontiguousarray(
            np.broadcast_to(b2[e].astype(np.float32), (P, D))
        )
        in_maps.append(
            {"xt": xt_e, "w1": w1_e, "b1": b1_e, "w2": w2_e, "b2": b2_e}
        )
    return in_maps


def kernel(inputs, w1, b1, w2, b2):
    global LAST_RESULT
    nc = _get_nc(T)
    in_maps = make_in_maps(inputs, w1, b1, w2, b2)
    res = run_bass_kernel_spmd(nc, in_maps, core_ids=list(range(N_CORES)))
    LAST_RESULT = res
    out = np.empty((B, EC, D), dtype=np.float32)
    for e in range(E):
        out[:, e * C:(e + 1) * C, :] = res.results[e]["y"].reshape(B, C, D)
    return out


# revision 11
# speedup vs baseline: 1.0037x; 1.0037x over previous
"""Expert-parallel MoE FFN kernel for Trainium2 (8 NeuronCores).

Reference computation (per expert e):
    y[:, e*C:(e+1)*C, :] = gelu(x_e @ w1[e] + b1[e]) @ w2[e] + b2[e]

Sharding: expert-parallel — core e owns expert e (E == n_cores == 8) and the
matching chunk of dim 1 of `inputs`. No cross-core communication.

Per-core dataflow (T=16384 tokens, D=512, F=2048), all matmuls bf16:
  - X is pre-transposed and pre-cast to bf16 on the HOST into
    [128d, nchunk, DSUB, 512t] so the device streams X^T tiles straight from
    DRAM (no on-device cast, no DRAM bounce, no XBAR transpose). This pulls
    the first matmul from t=35us down to the preamble floor and removes
    ~48 MiB/core of HBM traffic.
  - mm1: H^T[f, t] += W1[d, f].T @ X^T[d, t]; gelu+b1 fused on ScalarE
    (f on partitions -> b1 is a per-partition bias), H stored bf16.
  - mm2: Y[t, d] += (H^T[f, t128]).T @ W2[f, d] with H^T as the stationary
    operand, so Y comes out token-major and stores contiguously.
"""

import numpy as np
import ml_dtypes

import concourse.bacc as bacc
import concourse.bass as bass
import concourse.mybir as mybir
import concourse.tile as tile
from concourse.bass_utils import run_bass_kernel_spmd

B, EC, D = 16, 8192, 512
E, F = 8, 2048
C = EC // E            # capacity per expert = 1024
T = B * C              # tokens per expert/core = 16384
P = 128
DSUB = D // P          # 4
FSUB = F // P          # 16
TCHUNK = 512
TS = TCHUNK // P       # 4
NCHUNK = T // TCHUNK   # 32
N_CORES = 8

# Stash of the last BassKernelResults (for test harness profiling).
LAST_RESULT = None


def build_nc(n_tokens: int = T, act_func=None):
    if act_func is None:
        act_func = mybir.ActivationFunctionType.Gelu_apprx_tanh
    nchunk = n_tokens // TCHUNK
    nc = bacc.Bacc(
        "TRN2",
        target_bir_lowering=False,
        debug=False,
        num_devices=N_CORES,
    )
    # Host-pre-transposed X^T: xt[p, c, ds, j] = x[c*512 + j, ds*128 + p], bf16
    xt_d = nc.dram_tensor(
        "xt", [P, nchunk, DSUB, TCHUNK], mybir.dt.bfloat16, kind="ExternalInput"
    ).ap()
    # fs-major w1 so each 128x128 lhsT tile is contiguous per partition and
    # the first f-tile can be DMA'd ahead of the bulk.
    w1 = nc.dram_tensor(
        "w1", [P, FSUB, DSUB, P], mybir.dt.bfloat16, kind="ExternalInput"
    ).ap()
    b1 = nc.dram_tensor("b1", [P, FSUB], mybir.dt.float32, kind="ExternalInput").ap()
    w2 = nc.dram_tensor("w2", [P, FSUB, D], mybir.dt.bfloat16, kind="ExternalInput").ap()
    b2 = nc.dram_tensor("b2", [P, D], mybir.dt.float32, kind="ExternalInput").ap()
    y = nc.dram_tensor("y", [n_tokens, D], mybir.dt.float32, kind="ExternalOutput").ap()

    with tile.TileContext(nc) as tc:
        with (
            tc.tile_pool(name="consts", bufs=1) as consts,
            tc.tile_pool(name="xt", bufs=4) as xt_pool,
            tc.tile_pool(name="h", bufs=2) as h_pool,
            tc.tile_pool(name="yout", bufs=4) as y_pool,
            tc.tile_pool(name="ps_h", bufs=4, space="PSUM") as ps_h,
            tc.tile_pool(name="ps_y", bufs=4, space="PSUM") as ps_y,
        ):
            # Spread const loads over parallel HWDGE queues, one DMA per w1
            # f-tile so each tile's completion sem fires as soon as mm1 needs
            # it (coarse chunks made mm1 of chunk 0 stall on the bulk):
            #   scalar: b1 + w1 fs0-3 (then the queue is free for ACTIVATEs)
            #   gpsimd: b2 + w1 fs4-15 + w2
            #   sync:   X^T chunk stream + Y stores.
            b1_sb = consts.tile([P, FSUB], mybir.dt.float32)
            nc.scalar.dma_start(b1_sb, b1)
            w1_sb = consts.tile([P, FSUB, DSUB, P], mybir.dt.bfloat16)
            for fs in range(4):
                nc.scalar.dma_start(w1_sb[:, fs], w1[:, fs])
            b2_sb = consts.tile([P, D], mybir.dt.float32)
            nc.gpsimd.dma_start(b2_sb, b2)
            for fs in range(4, FSUB):
                nc.gpsimd.dma_start(w1_sb[:, fs], w1[:, fs])
            w2_sb = consts.tile([P, FSUB, D], mybir.dt.bfloat16)
            for i in range(4):
                nc.gpsimd.dma_start(
                    w2_sb[:, i * 4:(i + 1) * 4, :], w2[:, i * 4:(i + 1) * 4, :]
                )

            for c in range(nchunk):
                # X^T [d, t] bf16 straight from DRAM (4 KiB/partition, contig).
                xt = xt_pool.tile([P, DSUB, TCHUNK], mybir.dt.bfloat16)
                nc.sync.dma_start(xt, xt_d[:, c])

                # mm1 + fused gelu/bias: H^T[f, t] bf16.
                h = h_pool.tile([P, FSUB, TCHUNK], mybir.dt.bfloat16)
                for fs in range(FSUB):
                    ph = ps_h.tile([P, TCHUNK], mybir.dt.float32)
                    for ds in range(DSUB):
                        nc.tensor.matmul(
                            ph,
                            lhsT=w1_sb[:, fs, ds, :],
                            rhs=xt[:, ds, :],
                            start=(ds == 0),
                            stop=(ds == DSUB - 1),
                        )
                    nc.scalar.activation(
                        h[:, fs, :],
                        ph,
                        act_func,
                        bias=b1_sb[:, fs:fs + 1],
                        scale=1.0,
                    )

                # mm2: Y[t, d] per 128-token subtile; + b2; store.
                for ts in range(TS):
                    py = ps_y.tile([P, D], mybir.dt.float32)
                    for fs in range(FSUB):
                        nc.tensor.matmul(
                            py,
                            lhsT=h[:, fs, ts * P:(ts + 1) * P],
                            rhs=w2_sb[:, fs, :],
                            start=(fs == 0),
                            stop=(fs == FSUB - 1),
                        )
                    y_sb = y_pool.tile([P, D], mybir.dt.float32)
                    nc.vector.tensor_add(y_sb, py, b2_sb)
                    r0 = c * TCHUNK + ts * P
                    nc.sync.dma_start(y[r0:r0 + P, :], y_sb)

    nc.compile()
    return nc


_NC_CACHE = {}


def _get_nc(n_tokens: int = T):
    if n_tokens not in _NC_CACHE:
        _NC_CACHE[n_tokens] = build_nc(n_tokens)
    return _NC_CACHE[n_tokens]


def make_in_maps(inputs, w1, b1, w2, b2):
    """Shard + lay out host-side: core e gets expert e."""
    bf16 = ml_dtypes.bfloat16
    inputs = np.asarray(inputs)
    w1, b1 = np.asarray(w1), np.asarray(b1)
    w2, b2 = np.asarray(w2), np.asarray(b2)
    in_maps = []
    for e in range(E):
        x_e = np.asarray(
            inputs[:, e * C:(e + 1) * C, :], dtype=np.float32
        ).reshape(T, D)
        # X^T bf16: xt[p, c, ds, j] = x_e[c*TCHUNK + j, ds*128 + p]
        xt_e = np.ascontiguousarray(
            x_e.reshape(NCHUNK, TCHUNK, DSUB, P).transpose(3, 0, 2, 1).astype(bf16)
        )
        # w1[e] [D, F] -> [P, FSUB, DSUB, P] with d = ds*128 + p, f = fs*128 + j
        w1_e = np.ascontiguousarray(
            w1[e].reshape(DSUB, P, FSUB, P).transpose(1, 2, 0, 3).astype(bf16)
        )
        # b1[e] [F] -> [P, FSUB] with f = fs*128 + p
        b1_e = np.ascontiguousarray(
            b1[e].reshape(FSUB, P).T.astype(np.float32)
        )
        # w2[e] [F, D] -> [P, FSUB, D] with f = fs*128 + p
        w2_e = np.ascontiguousarray(
            w2[e].reshape(FSUB, P, D).transpose(1, 0, 2).astype(bf16)
        )
        # b2[e] [D] -> broadcast to [P, D]
        b2_e = np.ascontiguousarray(
            np.broadcast_to(b2[e].astype(np.float32), (P, D))
        )
        in_maps.append(
            {"xt": xt_e, "w1": w1_e, "b1": b1_e, "w2": w2_e, "b2": b2_e}
        )
    return in_maps


def kernel(inputs, w1, b1, w2, b2):
    global LAST_RESULT
    nc = _get_nc(T)
    in_maps = make_in_maps(inputs, w1, b1, w2, b2)
    res = run_bass_kernel_spmd(nc, in_maps, core_ids=list(range(N_CORES)))
    LAST_RESULT = res
    out = np.empty((B, EC, D), dtype=np.float32)
    for e in range(E):
        out[:, e * C:(e + 1) * C, :] = res.results[e]["y"].reshape(B, C, D)
    return out


# revision 13
# speedup vs baseline: 1.0078x; 1.0041x over previous
"""Expert-parallel MoE FFN kernel for Trainium2 (8 NeuronCores).

Reference computation (per expert e):
    y[:, e*C:(e+1)*C, :] = gelu(x_e @ w1[e] + b1[e]) @ w2[e] + b2[e]

Sharding: expert-parallel — core e owns expert e (E == n_cores == 8) and the
matching chunk of dim 1 of `inputs`. No cross-core communication.

Per-core dataflow (T=16384 tokens, D=512, F=2048), all matmuls bf16:
  - X is pre-transposed and pre-cast to bf16 on the HOST into
    [128d, nchunk, DSUB, 512t] so the device streams X^T tiles straight from
    DRAM (no on-device cast, no DRAM bounce, no XBAR transpose). This pulls
    the first matmul from t=35us down to the preamble floor and removes
    ~48 MiB/core of HBM traffic.
  - mm1: H^T[f, t] += W1[d, f].T @ X^T[d, t]; gelu+b1 fused on ScalarE
    (f on partitions -> b1 is a per-partition bias), H stored bf16.
  - mm2: Y[t, d] += (H^T[f, t128]).T @ W2[f, d] with H^T as the stationary
    operand, so Y comes out token-major and stores contiguously.
"""

import numpy as np
import ml_dtypes

import concourse.bacc as bacc
import concourse.bass as bass
import concourse.mybir as mybir
import concourse.tile as tile
from concourse.bass_utils import run_bass_kernel_spmd

B, EC, D = 16, 8192, 512
E, F = 8, 2048
C = EC // E            # capacity per expert = 1024
T = B * C              # tokens per expert/core = 16384
P = 128
DSUB = D // P          # 4
FSUB = F // P          # 16
TCHUNK = 512
TS = TCHUNK // P       # 4
NCHUNK = T // TCHUNK   # 32
N_CORES = 8

# Stash of the last BassKernelResults (for test harness profiling).
LAST_RESULT = None


def build_nc(n_tokens: int = T, act_func=None):
    if act_func is None:
        act_func = mybir.ActivationFunctionType.Gelu_apprx_tanh
    nchunk = n_tokens // TCHUNK
    nc = bacc.Bacc(
        "TRN2",
        target_bir_lowering=False,
        debug=False,
        num_devices=N_CORES,
    )
    # Host-pre-transposed X^T: xt[p, c, ds, j] = x[c*512 + j, ds*128 + p], bf16
    xt_d = nc.dram_tensor(
        "xt", [P, nchunk, DSUB, TCHUNK], mybir.dt.bfloat16, kind="ExternalInput"
    ).ap()
    # fs-major w1 so each 128x128 lhsT tile is contiguous per partition and
    # the first f-tile can be DMA'd ahead of the bulk.
    w1 = nc.dram_tensor(
        "w1", [P, FSUB, DSUB, P], mybir.dt.bfloat16, kind="ExternalInput"
    ).ap()
    b1 = nc.dram_tensor("b1", [P, FSUB], mybir.dt.float32, kind="ExternalInput").ap()
    w2 = nc.dram_tensor("w2", [P, FSUB, D], mybir.dt.bfloat16, kind="ExternalInput").ap()
    b2 = nc.dram_tensor("b2", [P, D], mybir.dt.float32, kind="ExternalInput").ap()
    y = nc.dram_tensor("y", [n_tokens, D], mybir.dt.float32, kind="ExternalOutput").ap()

    with tile.TileContext(nc) as tc:
        with (
            tc.tile_pool(name="consts", bufs=1) as consts,
            tc.tile_pool(name="xt", bufs=2) as xt_pool,
            tc.tile_pool(name="h", bufs=2) as h_pool,
            tc.tile_pool(name="yout", bufs=4) as y_pool,
            tc.tile_pool(name="ps_h", bufs=4, space="PSUM") as ps_h,
            tc.tile_pool(name="ps_y", bufs=4, space="PSUM") as ps_y,
        ):
            # Spread const loads over parallel HWDGE queues, one DMA per w1
            # f-tile so each tile's completion sem fires as soon as mm1 needs
            # it (coarse chunks made mm1 of chunk 0 stall on the bulk):
            #   scalar: b1 + w1 fs0-3 (then the queue is free for ACTIVATEs)
            #   gpsimd: b2 + w1 fs4-15 + w2
            #   sync:   X^T chunk stream + Y stores.
            b1_sb = consts.tile([P, FSUB], mybir.dt.float32)
            nc.scalar.dma_start(b1_sb, b1)
            w1_sb = consts.tile([P, FSUB, DSUB, P], mybir.dt.bfloat16)
            for fs in range(0, 6, 2):
                nc.scalar.dma_start(w1_sb[:, fs:fs + 2], w1[:, fs:fs + 2])
            for fs in range(6, FSUB, 2):
                nc.gpsimd.dma_start(w1_sb[:, fs:fs + 2], w1[:, fs:fs + 2])
            b2_sb = consts.tile([P, D], mybir.dt.float32)
            nc.gpsimd.dma_start(b2_sb, b2)
            w2_sb = consts.tile([P, FSUB, D], mybir.dt.bfloat16)
            for i in range(4):
                nc.gpsimd.dma_start(
                    w2_sb[:, i * 4:(i + 1) * 4, :], w2[:, i * 4:(i + 1) * 4, :]
                )

            for c in range(nchunk):
                # X^T [d, t] bf16 straight from DRAM (4 KiB/partition, contig).
                xt = xt_pool.tile([P, DSUB, TCHUNK], mybir.dt.bfloat16)
                nc.sync.dma_start(xt, xt_d[:, c])

                # mm1 + fused gelu/bias: H^T[f, t] bf16.
                h = h_pool.tile([P, FSUB, TCHUNK], mybir.dt.bfloat16)
                for fs in range(FSUB):
                    ph = ps_h.tile([P, TCHUNK], mybir.dt.float32)
                    for ds in range(DSUB):
                        nc.tensor.matmul(
                            ph,
                            lhsT=w1_sb[:, fs, ds, :],
                            rhs=xt[:, ds, :],
                            start=(ds == 0),
                            stop=(ds == DSUB - 1),
                        )
                    nc.scalar.activation(
                        h[:, fs, :],
                        ph,
                        act_func,
                        bias=b1_sb[:, fs:fs + 1],
                        scale=1.0,
                    )

                # mm2: Y[t, d] per 128-token subtile; + b2; store.
                for ts in range(TS):
                    py = ps_y.tile([P, D], mybir.dt.float32)
                    for fs in range(FSUB):
                        nc.tensor.matmul(
                            py,
                            lhsT=h[:, fs, ts * P:(ts + 1) * P],
                            rhs=w2_sb[:, fs, :],
                            start=(fs == 0),
                            stop=(fs == FSUB - 1),
                        )
                    y_sb = y_pool.tile([P, D], mybir.dt.float32)
                    nc.vector.tensor_add(y_sb, py, b2_sb)
                    r0 = c * TCHUNK + ts * P
                    nc.sync.dma_start(y[r0:r0 + P, :], y_sb)

    nc.compile()
    return nc


_NC_CACHE = {}


def _get_nc(n_tokens: int = T):
    if n_tokens not in _NC_CACHE:
        _NC_CACHE[n_tokens] = build_nc(n_tokens)
    return _NC_CACHE[n_tokens]


def make_in_maps(inputs, w1, b1, w2, b2):
    """Shard + lay out host-side: core e gets expert e."""
    bf16 = ml_dtypes.bfloat16
    inputs = np.asarray(inputs)
    w1, b1 = np.asarray(w1), np.asarray(b1)
    w2, b2 = np.asarray(w2), np.asarray(b2)
    in_maps = []
    for e in range(E):
        x_e = np.asarray(
            inputs[:, e * C:(e + 1) * C, :], dtype=np.float32
        ).reshape(T, D)
        # X^T bf16: xt[p, c, ds, j] = x_e[c*TCHUNK + j, ds*128 + p]
        xt_e = np.ascontiguousarray(
            x_e.reshape(NCHUNK, TCHUNK, DSUB, P).transpose(3, 0, 2, 1).astype(bf16)
        )
        # w1[e] [D, F] -> [P, FSUB, DSUB, P] with d = ds*128 + p, f = fs*128 + j
        w1_e = np.ascontiguousarray(
            w1[e].reshape(DSUB, P, FSUB, P).transpose(1, 2, 0, 3).astype(bf16)
        )
        # b1[e] [F] -> [P, FSUB] with f = fs*128 + p
        b1_e = np.ascontiguousarray(
            b1[e].reshape(FSUB, P).T.astype(np.float32)
        )
        # w2[e] [F, D] -> [P, FSUB, D] with f = fs*128 + p
        w2_e = np.ascontiguousarray(
            w2[e].reshape(FSUB, P, D).transpose(1, 0, 2).astype(bf16)
        )
        # b2[e] [D] -> broadcast to [P, D]
        b2_e = np.ascontiguousarray(
            np.broadcast_to(b2[e].astype(np.float32), (P, D))
        )
        in_maps.append(
            {"xt": xt_e, "w1": w1_e, "b1": b1_e, "w2": w2_e, "b2": b2_e}
        )
    return in_maps


def kernel(inputs, w1, b1, w2, b2):
    global LAST_RESULT
    nc = _get_nc(T)
    in_maps = make_in_maps(inputs, w1, b1, w2, b2)
    res = run_bass_kernel_spmd(nc, in_maps, core_ids=list(range(N_CORES)))
    LAST_RESULT = res
    out = np.empty((B, EC, D), dtype=np.float32)
    for e in range(E):
        out[:, e * C:(e + 1) * C, :] = res.results[e]["y"].reshape(B, C, D)
    return out


# revision 15
# speedup vs baseline: 1.0118x; 1.0040x over previous
"""Expert-parallel MoE FFN kernel for Trainium2 (8 NeuronCores).

Reference computation (per expert e):
    y[:, e*C:(e+1)*C, :] = gelu(x_e @ w1[e] + b1[e]) @ w2[e] + b2[e]

Sharding: expert-parallel — core e owns expert e (E == n_cores == 8) and the
matching chunk of dim 1 of `inputs`. No cross-core communication.

Per-core dataflow (T=16384 tokens, D=512, F=2048), all matmuls bf16:
  - X is pre-transposed and pre-cast to bf16 on the HOST into
    [128d, nchunk, DSUB, 512t] so the device streams X^T tiles straight from
    DRAM (no on-device cast, no DRAM bounce, no XBAR transpose). This pulls
    the first matmul from t=35us down to the preamble floor and removes
    ~48 MiB/core of HBM traffic.
  - mm1: H^T[f, t] += W1[d, f].T @ X^T[d, t]; gelu+b1 fused on ScalarE
    (f on partitions -> b1 is a per-partition bias), H stored bf16.
  - mm2: Y[t, d] += (H^T[f, t128]).T @ W2[f, d] with H^T as the stationary
    operand, so Y comes out token-major and stores contiguously.
"""

import numpy as np
import ml_dtypes

import concourse.bacc as bacc
import concourse.bass as bass
import concourse.mybir as mybir
import concourse.tile as tile
from concourse.bass_utils import run_bass_kernel_spmd

B, EC, D = 16, 8192, 512
E, F = 8, 2048
C = EC // E            # capacity per expert = 1024
T = B * C              # tokens per expert/core = 16384
P = 128
DSUB = D // P          # 4
FSUB = F // P          # 16
TCHUNK = 512
TS = TCHUNK // P       # 4
NCHUNK = T // TCHUNK   # 32
N_CORES = 8

# Stash of the last BassKernelResults (for test harness profiling).
LAST_RESULT = None


def build_nc(n_tokens: int = T, act_func=None):
    if act_func is None:
        act_func = mybir.ActivationFunctionType.Gelu_apprx_tanh
    nchunk = n_tokens // TCHUNK
    nc = bacc.Bacc(
        "TRN2",
        target_bir_lowering=False,
        debug=False,
        num_devices=N_CORES,
    )
    # Host-pre-transposed X^T: xt[p, c, ds, j] = x[c*512 + j, ds*128 + p], bf16
    xt_d = nc.dram_tensor(
        "xt", [P, nchunk, DSUB, TCHUNK], mybir.dt.bfloat16, kind="ExternalInput"
    ).ap()
    # fs-major w1 so each 128x128 lhsT tile is contiguous per partition and
    # the first f-tile can be DMA'd ahead of the bulk.
    w1 = nc.dram_tensor(
        "w1", [P, FSUB, DSUB, P], mybir.dt.bfloat16, kind="ExternalInput"
    ).ap()
    b1 = nc.dram_tensor("b1", [P, FSUB], mybir.dt.float32, kind="ExternalInput").ap()
    w2 = nc.dram_tensor("w2", [P, FSUB, D], mybir.dt.bfloat16, kind="ExternalInput").ap()
    b2 = nc.dram_tensor("b2", [P, D], mybir.dt.float32, kind="ExternalInput").ap()
    y = nc.dram_tensor("y", [n_tokens, D], mybir.dt.float32, kind="ExternalOutput").ap()

    with tile.TileContext(nc) as tc:
        with (
            tc.tile_pool(name="consts", bufs=1) as consts,
            tc.tile_pool(name="xt", bufs=2) as xt_pool,
            tc.tile_pool(name="h", bufs=2) as h_pool,
            tc.tile_pool(name="yout", bufs=4) as y_pool,
            tc.tile_pool(name="ps_h", bufs=4, space="PSUM") as ps_h,
            tc.tile_pool(name="ps_y", bufs=4, space="PSUM") as ps_y,
        ):
            # Every [128, *] SBUF load is >=128 descriptors at ~25ns each, so
            # each DMA has a ~3us floor and each queue streams ~85-170 GB/s.
            # mm1 consumes one 128KB w1 f-tile per 853ns from ~10.5us on, so
            # alternate w1 f-tile PAIRS (2KB descriptors) across the scalar
            # and gpsimd queues to deliver ~1.33 tiles/us, then split w2 the
            # same way. b1/b2 ride the sync queue behind xt chunk 0 (they
            # aren't needed until the first ACTIVATE / first mm2 add).
            b1_sb = consts.tile([P, FSUB], mybir.dt.float32)
            b2_sb = consts.tile([P, D], mybir.dt.float32)
            w1_sb = consts.tile([P, FSUB, DSUB, P], mybir.dt.bfloat16)
            for fs in range(0, FSUB, 2):
                q = nc.scalar if (fs // 2) % 2 == 0 else nc.gpsimd
                q.dma_start(w1_sb[:, fs:fs + 2], w1[:, fs:fs + 2])
            w2_sb = consts.tile([P, FSUB, D], mybir.dt.bfloat16)
            for i in range(4):
                q = nc.scalar if i % 2 == 0 else nc.gpsimd
                q.dma_start(
                    w2_sb[:, i * 4:(i + 1) * 4, :], w2[:, i * 4:(i + 1) * 4, :]
                )

            for c in range(nchunk):
                # X^T [d, t] bf16 straight from DRAM (4 KiB/partition, contig).
                xt = xt_pool.tile([P, DSUB, TCHUNK], mybir.dt.bfloat16)
                nc.sync.dma_start(xt, xt_d[:, c])
                if c == 0:
                    # Biases queue on sync right behind xt chunk 0.
                    nc.sync.dma_start(b1_sb, b1)
                    nc.sync.dma_start(b2_sb, b2)

                # mm1 + fused gelu/bias: H^T[f, t] bf16.
                h = h_pool.tile([P, FSUB, TCHUNK], mybir.dt.bfloat16)
                for fs in range(FSUB):
                    ph = ps_h.tile([P, TCHUNK], mybir.dt.float32)
                    for ds in range(DSUB):
                        nc.tensor.matmul(
                            ph,
                            lhsT=w1_sb[:, fs, ds, :],
                            rhs=xt[:, ds, :],
                            start=(ds == 0),
                            stop=(ds == DSUB - 1),
                        )
                    nc.scalar.activation(
                        h[:, fs, :],
                        ph,
                        act_func,
                        bias=b1_sb[:, fs:fs + 1],
                        scale=1.0,
                    )

                # mm2: Y[t, d] per 128-token subtile; + b2; store.
                for ts in range(TS):
                    py = ps_y.tile([P, D], mybir.dt.float32)
                    for fs in range(FSUB):
                        nc.tensor.matmul(
                            py,
                            lhsT=h[:, fs, ts * P:(ts + 1) * P],
                            rhs=w2_sb[:, fs, :],
                            start=(fs == 0),
                            stop=(fs == FSUB - 1),
                        )
                    y_sb = y_pool.tile([P, D], mybir.dt.float32)
                    nc.vector.tensor_add(y_sb, py, b2_sb)
                    r0 = c * TCHUNK + ts * P
                    nc.sync.dma_start(y[r0:r0 + P, :], y_sb)

    nc.compile()
    return nc


_NC_CACHE = {}


def _get_nc(n_tokens: int = T):
    if n_tokens not in _NC_CACHE:
        _NC_CACHE[n_tokens] = build_nc(n_tokens)
    return _NC_CACHE[n_tokens]


def make_in_maps(inputs, w1, b1, w2, b2):
    """Shard + lay out host-side: core e gets expert e."""
    bf16 = ml_dtypes.bfloat16
    inputs = np.asarray(inputs)
    w1, b1 = np.asarray(w1), np.asarray(b1)
    w2, b2 = np.asarray(w2), np.asarray(b2)
    in_maps = []
    for e in range(E):
        x_e = np.asarray(
            inputs[:, e * C:(e + 1) * C, :], dtype=np.float32
        ).reshape(T, D)
        # X^T bf16: xt[p, c, ds, j] = x_e[c*TCHUNK + j, ds*128 + p]
        xt_e = np.ascontiguousarray(
            x_e.reshape(NCHUNK, TCHUNK, DSUB, P).transpose(3, 0, 2, 1).astype(bf16)
        )
        # w1[e] [D, F] -> [P, FSUB, DSUB, P] with d = ds*128 + p, f = fs*128 + j
        w1_e = np.ascontiguousarray(
            w1[e].reshape(DSUB, P, FSUB, P).transpose(1, 2, 0, 3).astype(bf16)
        )
        # b1[e] [F] -> [P, FSUB] with f = fs*128 + p
        b1_e = np.ascontiguousarray(
            b1[e].reshape(FSUB, P).T.astype(np.float32)
        )
        # w2[e] [F, D] -> [P, FSUB, D] with f = fs*128 + p
        w2_e = np.ascontiguousarray(
            w2[e].reshape(FSUB, P, D).transpose(1, 0, 2).astype(bf16)
        )
        # b2[e] [D] -> broadcast to [P, D]
        b2_e = np.ascontiguousarray(
            np.broadcast_to(b2[e].astype(np.float32), (P, D))
        )
        in_maps.append(
            {"xt": xt_e, "w1": w1_e, "b1": b1_e, "w2": w2_e, "b2": b2_e}
        )
    return in_maps


def kernel(inputs, w1, b1, w2, b2):
    global LAST_RESULT
    nc = _get_nc(T)
    in_maps = make_in_maps(inputs, w1, b1, w2, b2)
    res = run_bass_kernel_spmd(nc, in_maps, core_ids=list(range(N_CORES)))
    LAST_RESULT = res
    out = np.empty((B, EC, D), dtype=np.float32)
    for e in range(E):
        out[:, e * C:(e + 1) * C, :] = res.results[e]["y"].reshape(B, C, D)
    return out
